# revision 15
# baseline (speedup 1.0000x reference)
"""Deformable-conv layer kernel for 8 Trainium2 NeuronCores (Bass/Tile).

kernel(**inputs): full inputs -> full output [2,48,48,24] f32.
Data parallel over (batch, H/4) -> 8 shards of 576 pixels.

Per core (576 pixels s, 216 sample-channels k):
  G[y,k,c]  = sum_ch M[y*48+c, ch] * k4[k, ch]       (PE 1x1-conv compress)
  one stream_shuffle replicates y_k to rows 0-63 and x_k to rows 64-127,
  one Abs + one Relu build both tent (bilinear-weight) matrices:
  Ry[r, s]  = tent(y_k[s] - r), Cx[c, s] = tent(x_k[s] - c)
  T_k       = G_k^T @ Ry   -> [48c x 576s]           (PE, N=512+64 split)
  P_k       = T_k * Cx_k                             (DVE, one op)
  out_u    += ones^T @ P_k  (partition sum + 9-tap PSUM accumulation)
tent(d) = relu(1-|d|) equals the reference bilinear weights exactly; the
y==47 / x==47 clip corner (reference weights all zero there) is handled
by adding 1e6 to the coordinate so the tents vanish.
"""

import sys

for _p in ("/opt/trn_rl_repo",):
    if _p not in sys.path:
        sys.path.insert(0, _p)

import numpy as np

B, H, W, C = 2, 48, 48, 32
U = 24
KH = KW = 3
PAD = 1
K = KH * KW * U          # 216
NCORES = 8
HLOC = H // 4            # 12
S = HLOC * W             # 576
DY = 48                  # y domain (full; border samples have base 0)
DX = 48                  # x domain
BIG = 1.0e6

_PROG = {}


def _base_grids():
    hh = np.arange(H)[:, None, None, None] + np.arange(KH)[None, None, :, None] - PAD
    ww = np.arange(W)[None, :, None, None] + np.arange(KW)[None, None, None, :] - PAD
    hh = np.broadcast_to(hh, (H, W, KH, KW))
    ww = np.broadcast_to(ww, (H, W, KH, KW))
    valid = (hh >= 0) & (hh < H) & (ww >= 0) & (ww < W)
    yb = np.where(valid, hh, 0).reshape(H, W, 9).astype(np.float32)
    xb = np.where(valid, ww, 0).reshape(H, W, 9).astype(np.float32)
    return yb, xb


def _build_program():
    import concourse.mybir as mybir
    import concourse.tile as tile
    from concourse import bacc

    f32 = mybir.dt.float32
    Alu = mybir.AluOpType
    Act = mybir.ActivationFunctionType

    nc = bacc.Bacc("TRN2", target_bir_lowering=False, debug=False)

    d_mtw = nc.declare_dram_parameter("mtw", [C, H * W], f32, isOutput=False)
    d_kct = nc.declare_dram_parameter("kct", [C, K], f32, isOutput=False)
    d_offy = nc.declare_dram_parameter("offy", [K, S], f32, isOutput=False)
    d_offx = nc.declare_dram_parameter("offx", [K, S], f32, isOutput=False)
    d_ybf = nc.declare_dram_parameter("ybf", [K, S], f32, isOutput=False)
    d_xbf = nc.declare_dram_parameter("xbf", [K, S], f32, isOutput=False)
    # consts [128, 1]: -iota (rows 0-63; junk above 48 is harmless)
    d_cst = nc.declare_dram_parameter("cst", [128, 1], f32, isOutput=False)
    d_ones = nc.declare_dram_parameter("ones48", [48, 1], f32, isOutput=False)
    d_bias = nc.declare_dram_parameter("biasr", [1, U], f32, isOutput=False)
    d_out = nc.declare_dram_parameter("out", [1, U * S], f32, isOutput=True)

    NB = (K + 31) // 32                 # 7 blocks of 32 k
    NT = (K + 63) // 64                 # 4 prep tiles of 64 k

    with tile.TileContext(nc) as tc:
        with (
            tc.tile_pool(name="persist", bufs=1) as pp,
            tc.tile_pool(name="work", bufs=3) as wp,
            tc.tile_pool(name="psA", bufs=2, space="PSUM") as psA,
            tc.tile_pool(name="psB", bufs=1, space="PSUM") as psB,
            tc.tile_pool(name="psG", bufs=1, space="PSUM") as psG,
        ):
            mtw0 = pp.tile([C, H * W], f32, tag="mtw0", name="mtw0")
            nc.sync.dma_start(out=mtw0[:], in_=d_mtw[:])
            kct0 = pp.tile([C, K], f32, tag="kct0", name="kct0")
            nc.sync.dma_start(out=kct0[:], in_=d_kct[:])
            # PE operands go through DVE copies so Matmult instructions
            # wait on engine semaphores only (walrus chokes on multiple
            # DMA-lane waits attached to one LdWeights).
            mtw = pp.tile([C, H * W], f32, tag="mtw", name="mtw")
            nc.vector.tensor_copy(out=mtw[:], in_=mtw0[:])
            kct = pp.tile([C, K], f32, tag="kct", name="kct")
            nc.vector.tensor_copy(out=kct[:], in_=kct0[:])
            cst = pp.tile([128, 1], f32, tag="cst", name="cst")
            nc.sync.dma_start(out=cst[:], in_=d_cst[:])
            ones0 = pp.tile([48, 1], f32, tag="ones0", name="ones0")
            nc.sync.dma_start(out=ones0[:], in_=d_ones[:])
            ones48 = pp.tile([48, 1], f32, tag="ones48", name="ones48")
            nc.vector.tensor_copy(out=ones48[:], in_=ones0[:])
            biasr = pp.tile([1, U], f32, tag="biasr", name="biasr")
            nc.sync.dma_start(out=biasr[:], in_=d_bias[:])

            niota = cst[:, 0:1]

            # ---------- prep: y = fix(clip(ybf + offy)) in [64k x S] tiles
            prep = {}
            for side, (d_off, d_bg) in enumerate(
                ((d_offy, d_ybf), (d_offx, d_xbf))
            ):
                for t in range(NT):
                    r = min(64, K - 64 * t)
                    raw = wp.tile([r, S], f32, tag="raw", name=f"raw{side}{t}")
                    nc.sync.dma_start(out=raw[:], in_=d_off[64 * t : 64 * t + r, :])
                    bg = wp.tile([r, S], f32, tag="bg", name=f"bg{side}{t}")
                    nc.sync.dma_start(out=bg[:], in_=d_bg[64 * t : 64 * t + r, :])
                    yt = pp.tile([r, S], f32, tag=f"prep{side}{t}", name=f"prep{side}{t}")
                    nc.vector.tensor_tensor(out=yt[:], in0=raw[:], in1=bg[:], op=Alu.add)
                    nc.gpsimd.tensor_scalar(
                        out=yt[:], in0=yt[:], scalar1=0.0, scalar2=47.0,
                        op0=Alu.max, op1=Alu.min,
                    )
                    ee = wp.tile([r, S], f32, tag="ee", name=f"ee{side}{t}")
                    nc.gpsimd.tensor_scalar(
                        out=ee[:], in0=yt[:], scalar1=47.0, scalar2=BIG,
                        op0=Alu.is_equal, op1=Alu.mult,
                    )
                    nc.vector.tensor_tensor(out=yt[:], in0=yt[:], in1=ee[:], op=Alu.add)
                    prep[(side, t)] = yt

            # paired replication sources: qtY2[b] = [y_b, y_b, y_{b+3}, y_{b+3}]
            # (taps k and k+96 share ksub=k%32; block(k+96)=block(k)+3), and
            # qtX2 likewise; qt6 = [y6, y6, x6, x6] for the ij=8 singles.
            qtY2, qtX2 = {}, {}
            for b in range(3):
                for side, store in ((0, qtY2), (1, qtX2)):
                    dst = pp.tile([128, S], f32, tag=f"q{side}{b}", name=f"q{side}{b}")
                    lo = prep[(side, b // 2)][32 * (b % 2) : 32 * (b % 2) + 32, :]
                    hi = prep[(side, (b + 3) // 2)][32 * ((b + 3) % 2) : 32 * ((b + 3) % 2) + 32, :]
                    nc.sync.dma_start(out=dst[0:32, :], in_=lo)
                    nc.sync.dma_start(out=dst[32:64, :], in_=lo)
                    nc.sync.dma_start(out=dst[64:96, :], in_=hi)
                    nc.sync.dma_start(out=dst[96:128, :], in_=hi)
                    store[b] = dst
            qt6 = pp.tile([128, S], f32, tag="qt6", name="qt6")
            y6 = prep[(0, 3)][0:24, :]
            x6 = prep[(1, 3)][0:24, :]
            nc.sync.dma_start(out=qt6[0:24, :], in_=y6)
            nc.sync.dma_start(out=qt6[32:56, :], in_=y6)
            nc.sync.dma_start(out=qt6[64:88, :], in_=x6)
            nc.sync.dma_start(out=qt6[96:120, :], in_=x6)
            # ---------- G build ----------
            # g2: block-diag lhsT per tap-pair (k, k+96), k<96. Pair block
            # pidx=k occupies cols 128k..128k+128: rows 0-47 = G_k (cols 0-47),
            # rows 64-111 = G_{k+96} (cols 64-111), zeros elsewhere.
            g2 = pp.tile([128, 96 * 128], f32, tag="g2", name="g2")
            nc.gpsimd.memset(g2[:], 0.0)
            g48_6 = pp.tile([DY, 24 * 48], f32, tag="g48_6", name="g48_6")
            for c in range(48):
                gp = psG.tile([DY, K], f32, tag="gp", name=f"gp{c}")
                nc.tensor.matmul(
                    out=gp[:], lhsT=mtw[:, c::48], rhs=kct[:],
                    start=True, stop=True,
                )
                nc.vector.tensor_copy(out=g2[0:48, c::128], in_=gp[:, 0:96])
                nc.vector.tensor_copy(
                    out=g2[64:112, 64 + c :: 128], in_=gp[:, 96:192]
                )
                nc.vector.tensor_copy(out=g48_6[:, c::48], in_=gp[:, 192:216])
            # ---------- main loop ----------
            for u in range(U):
                acc = psB.tile([1, S], f32, tag="acc", name=f"acc_{u}")
                pt2 = {}
                for jj in range(4):
                    k1 = jj * U + u              # pair (k1, k1+96), pidx=k1
                    b, ksub = divmod(k1, 32)
                    msk = [ksub] * 32

                    ry2 = wp.tile([128, S], f32, tag="ry2", name=f"ry2_{u}_{jj}")
                    nc.vector.stream_shuffle(out=ry2[:], in_=qtY2[b][:], mask=msk)
                    cx2 = wp.tile([128, S], f32, tag="cx2", name=f"cx2_{u}_{jj}")
                    nc.vector.stream_shuffle(out=cx2[:], in_=qtX2[b][:], mask=msk)
                    for t in (ry2, cx2):
                        nc.scalar.activation(
                            out=t[:], in_=t[:], func=Act.Abs,
                            bias=niota[0:128, :], scale=1.0,
                        )
                        nc.scalar.activation(
                            out=t[:], in_=t[:], func=Act.Relu, bias=1.0, scale=-1.0
                        )

                    ta2 = psA.tile([128, S], f32, tag="ta", name=f"ta_{u}_{jj}")
                    for lo, hi in ((0, 512), (512, S)):
                        nc.tensor.matmul(
                            out=ta2[:, lo:hi],
                            lhsT=g2[:, 128 * k1 : 128 * (k1 + 1)],
                            rhs=ry2[:, lo:hi],
                            start=True, stop=True,
                        )

                    ptA = wp.tile([48, S], f32, tag="ptA", name=f"ptA_{u}_{jj}")
                    nc.vector.tensor_tensor(
                        out=ptA[:], in0=ta2[0:48, :], in1=cx2[0:48, :], op=Alu.mult
                    )
                    ptB = wp.tile([48, S], f32, tag=f"ptB{jj}", name=f"ptB_{u}_{jj}")
                    nc.vector.tensor_tensor(
                        out=ptB[:], in0=ta2[64:112, :], in1=cx2[64:112, :], op=Alu.mult
                    )
                    pt2[jj] = ptB

                    for lo, hi in ((0, 512), (512, S)):
                        nc.tensor.matmul(
                            out=acc[:, lo:hi],
                            lhsT=ones48[:],
                            rhs=ptA[:, lo:hi],
                            start=(jj == 0), stop=False,
                        )
                for jj in range(4):  # deferred ij = 4..7 in order
                    for lo, hi in ((0, 512), (512, S)):
                        nc.tensor.matmul(
                            out=acc[:, lo:hi],
                            lhsT=ones48[:],
                            rhs=pt2[jj][:, lo:hi],
                            start=False, stop=False,
                        )
                # ij=8 single (block 6): compute early, consume at the end
                rep = wp.tile([128, S], f32, tag="rep", name=f"rep_{u}")
                nc.vector.stream_shuffle(out=rep[:], in_=qt6[:], mask=[u] * 32)
                nc.scalar.activation(
                    out=rep[:], in_=rep[:], func=Act.Abs,
                    bias=niota[0:128, :], scale=1.0,
                )
                nc.scalar.activation(
                    out=rep[:], in_=rep[:], func=Act.Relu, bias=1.0, scale=-1.0
                )
                ta1 = psA.tile([128, S], f32, tag="ta", name=f"ta1_{u}")
                for lo, hi in ((0, 512), (512, S)):
                    nc.tensor.matmul(
                        out=ta1[0:48, lo:hi],
                        lhsT=g48_6[:, 48 * u : 48 * (u + 1)],
                        rhs=rep[0:48, lo:hi],
                        start=True, stop=True,
                    )
                pt8 = wp.tile([48, S], f32, tag="pt8", name=f"pt8_{u}")
                nc.vector.tensor_tensor(
                    out=pt8[:], in0=ta1[0:48, :], in1=rep[64:112, :], op=Alu.mult
                )

                for lo, hi in ((0, 512), (512, S)):  # ij = 8
                    nc.tensor.matmul(
                        out=acc[:, lo:hi],
                        lhsT=ones48[:],
                        rhs=pt8[:, lo:hi],
                        start=False, stop=True,
                    )

                ot = wp.tile([1, S], f32, tag="ot", name=f"ot_{u}")
                nc.scalar.activation(
                    out=ot[:],
                    in_=acc[:],
                    func=Act.Identity,
                    bias=biasr[:, u : u + 1],
                    scale=1.0,
                )
                nc.sync.dma_start(out=d_out[:, u * S : (u + 1) * S], in_=ot[:])

    nc.compile()
    return nc


def _r0_for(h0):
    return min(max(h0 - 8, 0), H - DY)


def kernel(inputs, offset, kernel, bias):
    from concourse.bass_utils import run_bass_kernel_spmd

    inputs = np.asarray(inputs, np.float32)
    offset = np.asarray(offset, np.float32)
    kernel = np.asarray(kernel, np.float32)
    bias = np.asarray(bias, np.float32)

    if "nc" not in _PROG:
        _PROG["nc"] = _build_program()
    nc = _PROG["nc"]

    yb9, xb9 = _base_grids()
    k4 = kernel.reshape(9, U, C).reshape(K, C)
    kct = np.ascontiguousarray(k4.T)
    ones48 = np.ones((48, 1), np.float32)
    biasr = np.ascontiguousarray(bias.reshape(1, U))

    in_maps = []
    for core in range(NCORES):
        bb, hc = divmod(core, 4)
        h0 = hc * HLOC
        xpad = np.pad(inputs[bb], ((PAD, PAD), (PAD, PAD), (0, 0)))[:H, :W]
        mtw = np.ascontiguousarray(xpad.reshape(H * W, C).T)      # [32, 2304]
        osl = offset[bb, h0 : h0 + HLOC].reshape(S, K, 2)
        offy = np.ascontiguousarray(osl[:, :, 0].T)
        offx = np.ascontiguousarray(osl[:, :, 1].T)
        yb_s = yb9[h0 : h0 + HLOC].reshape(S, 9).T                # [9, S]
        xb_s = xb9[h0 : h0 + HLOC].reshape(S, 9).T
        ybf = np.ascontiguousarray(np.repeat(yb_s, U, axis=0))    # [216, S]
        xbf = np.ascontiguousarray(np.repeat(xb_s, U, axis=0))
        cstm = np.full((128, 1), 1.0e4, np.float32)
        cstm[0:48, 0] = -np.arange(48)
        cstm[64:112, 0] = -np.arange(48)
        in_maps.append(
            dict(mtw=mtw, kct=kct, offy=offy, offx=offx, ybf=ybf, xbf=xbf,
                 cst=cstm, ones48=ones48, biasr=biasr)
        )

    import os as _os
    _trace = bool(int(_os.environ.get("KERNEL_TRACE", "0")))
    res = run_bass_kernel_spmd(
        nc, in_maps, list(range(NCORES)), trace=_trace)
    _PROG["last_results"] = res

    out = np.empty((B, H, W, U), np.float32)
    for core in range(NCORES):
        bb, hc = divmod(core, 4)
        h0 = hc * HLOC
        o = res.results[core]["out"].reshape(U, HLOC, W)
        out[bb, h0 : h0 + HLOC] = o.transpose(1, 2, 0)
    return out

